# revision 9
# baseline (speedup 1.0000x reference)
"""Trainium2 Bass kernel for the CWICDense (conditional stripe matmul) module.

Problem (hardcoded shapes):
  x          [2, 512, 4096] f32    tokens T=1024, features I=4096
  W_kernel   [4096, 4096]   f32    viewed as [I, N=32 stripes, Q=128]
  thresholds [4096, 32]     f32
  mu         [4096]         f32    (structurally zero in this module)
  out_mu     [4096]         f32
  where      [2, 512]       bool   (unused by the reference computation)

  y[t, n*Q+q] = sum_i x_off[t,i] * (|x_off[t,i]| >= thresholds[i,n]) * W[i, n*Q+q]
                + out_mu[n*Q+q]

Sharding across 8 NeuronCores: 2-way data parallel over tokens (halves of
512) x 4-way tensor parallel over stripes (groups of 8 stripes = 1024 out
cols). Each core computes y_c [512, 1024].

Per-core device algorithm:
  - PE-transpose x_c to x^T [I on partitions, T free] (128x128 blocks).
  - a = |x^T| once (ACT Abs, exact fp32).
  - per (stripe n, k-tile): z = (a >= thr[:,n]) * x^T with exact fp32
    compares: fused scalar_tensor_tensor on DVE for most k-tiles,
    mask+multiply pair on GPSIMD for the rest.
  - PE matmul (float32r, N=512 moving) accumulating y^T[n-block] in PSUM
    over 32 k-tiles: acc += W[k,n].T @ z.
  - epilogue: ACT adds out_mu (per-partition bias in y^T layout),
    PE-transpose back to [token, outcol] tiles, DMA out.
"""

import sys

if "/opt/trn_rl_repo" not in sys.path:
    sys.path.insert(0, "/opt/trn_rl_repo")

import numpy as np

import concourse.bass as bass
import concourse.mybir as mybir
import concourse.tile as tile
from concourse import bacc, bass_utils
from concourse.masks import make_identity

# ---- problem constants -------------------------------------------------
B, S, I, N, Q = 2, 512, 4096, 32, 128
T = B * S                 # 1024 tokens
OUT = N * Q               # 4096
NCORES = 8
TOK_WAYS = 2              # token halves
GRP_WAYS = 4              # stripe groups
T_C = T // TOK_WAYS       # 512 tokens per core
NS = N // GRP_WAYS        # 8 stripes per core
OUT_C = NS * Q            # 1024 out cols per core
KT = I // 128             # 32 contraction tiles
P = 128

# z-production split within each stripe's 32 k-tiles: the first DVE_K run as
# one fused scalar_tensor_tensor on DVE; the rest run fully on GPSIMD as a
# mask (tensor_scalar is_ge) + multiply (tensor_tensor) pair.
DVE_K = 24

_CACHE = {}


def _build():
    f32 = mybir.dt.float32
    f32r = mybir.dt.float32r
    nc = bacc.Bacc("TRN2", target_bir_lowering=False, debug=False)

    x_d = nc.dram_tensor("x", [T_C, I], f32, kind="ExternalInput").ap()
    w_d = nc.dram_tensor("w", [I, OUT_C], f32, kind="ExternalInput").ap()
    thr_d = nc.dram_tensor("thr", [I, NS], f32, kind="ExternalInput").ap()
    mu_d = nc.dram_tensor("mu", [P, NS], f32, kind="ExternalInput").ap()
    y_d = nc.dram_tensor("y", [T_C, OUT_C], f32, kind="ExternalOutput").ap()

    # strided DRAM views for batched loads
    # x_v[p, c, i] = x[c*128+p, i]
    x_v = x_d.rearrange("(c p) i -> p c i", p=P)
    # w_v[p, k, c] = w[k*128+p, c]
    w_v = w_d.rearrange("(k p) c -> p k c", p=P)

    ge = mybir.AluOpType.is_ge
    mult = mybir.AluOpType.mult

    with tile.TileContext(nc) as tc:
        with (
            tc.tile_pool(name="const", bufs=1) as constp,
            tc.tile_pool(name="xT", bufs=KT) as xTp,
            tc.tile_pool(name="absa", bufs=KT) as ap_,
            tc.tile_pool(name="xnat", bufs=3) as xnatp,
            tc.tile_pool(name="w", bufs=2) as wp,
            tc.tile_pool(name="z", bufs=8) as zp,
            tc.tile_pool(name="m", bufs=3) as mp,
            tc.tile_pool(name="yT", bufs=2) as yTp,
            tc.tile_pool(name="ysb", bufs=2) as ysbp,
            tc.tile_pool(name="tps", bufs=3, space="PSUM") as tpsp,
            tc.tile_pool(name="acc", bufs=3, space="PSUM") as accp,
        ):
            ident = constp.tile([P, P], f32, tag="ident")
            make_identity(nc, ident[:])

            thr_sb = constp.tile([P, KT * NS], f32, tag="thr")
            for k in range(KT):
                nc.sync.dma_start(
                    thr_sb[:, k * NS:(k + 1) * NS], thr_d[k * P:(k + 1) * P, :]
                )
            mu_sb = constp.tile([P, NS], f32, tag="mu")
            nc.sync.dma_start(mu_sb[:], mu_d)

            # ---- phase A: x -> x^T (PE transpose), a = |x^T| ----------
            xT = []
            aT = []
            for k in range(KT):
                xn = xnatp.tile([P, T_C], f32, tag="xnat")
                nc.sync.dma_start(
                    xn[:].rearrange("p (c i) -> p c i", i=P),
                    x_v[:, :, k * P:(k + 1) * P],
                )
                xk = xTp.tile([P, T_C], f32, tag="xT")
                ps = tpsp.tile([P, T_C], f32, tag="tps")
                for c in range(T_C // P):
                    nc.tensor.transpose(
                        ps[:, c * P:(c + 1) * P], xn[:, c * P:(c + 1) * P],
                        ident[:],
                    )
                nc.scalar.copy(xk[:], ps[:])
                ak = ap_.tile([P, T_C], f32, tag="absa")
                nc.scalar.activation(
                    ak[:], xk[:], mybir.ActivationFunctionType.Abs
                )
                xT.append(xk)
                aT.append(ak)

            # ---- phase B: masked stripe matmuls -----------------------
            for n in range(NS):
                # whole W column panel for this stripe in one DMA:
                # wn[p, k*Q+q] = w[k*128+p, n*Q+q]
                wn = wp.tile([P, KT * Q], f32r, tag="w")
                nc.sync.dma_start(
                    wn[:].rearrange("p (k q) -> p k q", q=Q),
                    w_v[:, :, n * Q:(n + 1) * Q].bitcast(f32r),
                )
                acc = accp.tile([P, T_C], f32, tag="acc")
                for k in range(KT):
                    zt = zp.tile([P, T_C], f32r, tag="z")
                    thr_ap = thr_sb[:, k * NS + n:k * NS + n + 1]
                    if k < DVE_K:
                        nc.vector.scalar_tensor_tensor(
                            zt[:], aT[k][:], thr_ap, xT[k][:],
                            op0=ge, op1=mult,
                        )
                    else:
                        mt = mp.tile([P, T_C], f32, tag="m")
                        nc.gpsimd.tensor_scalar(
                            mt[:], aT[k][:], thr_ap, None, op0=ge,
                        )
                        nc.gpsimd.tensor_tensor(
                            zt[:], mt[:], xT[k][:], op=mult
                        )
                    nc.tensor.matmul(
                        acc[:],
                        wn[:, k * Q:(k + 1) * Q],
                        zt[:],
                        start=(k == 0),
                        stop=(k == KT - 1),
                    )
                # epilogue for stripe n: + out_mu (ACT), transpose, store
                yT = yTp.tile([P, T_C], f32, tag="yT")
                nc.scalar.activation(
                    yT[:], acc[:], mybir.ActivationFunctionType.Identity,
                    bias=mu_sb[:, n:n + 1],
                )
                ps2 = tpsp.tile([P, T_C], f32, tag="tps")
                for c in range(T_C // P):
                    nc.tensor.transpose(
                        ps2[:, c * P:(c + 1) * P], yT[:, c * P:(c + 1) * P],
                        ident[:],
                    )
                ysb = ysbp.tile([P, T_C], f32, tag="ysb")
                nc.scalar.copy(ysb[:], ps2[:])
                for c in range(T_C // P):
                    nc.sync.dma_start(
                        y_d[c * P:(c + 1) * P, n * Q:(n + 1) * Q],
                        ysb[:, c * P:(c + 1) * P],
                    )
    nc.compile()
    return nc


def _get_nc():
    if "nc" not in _CACHE:
        _CACHE["nc"] = _build()
    return _CACHE["nc"]


def _make_in_maps(x, W_kernel, thresholds, mu, out_mu):
    xf = np.ascontiguousarray(x, dtype=np.float32).reshape(T, I)
    xf = xf - np.asarray(mu, dtype=np.float32)[None, :]
    in_maps = []
    for core in range(NCORES):
        h, g = divmod(core, GRP_WAYS)
        mu_c = np.ascontiguousarray(
            np.asarray(out_mu, dtype=np.float32)[g * OUT_C:(g + 1) * OUT_C]
            .reshape(NS, P).T
        )
        in_maps.append({
            "x": np.ascontiguousarray(xf[h * T_C:(h + 1) * T_C]),
            "w": np.ascontiguousarray(
                np.asarray(W_kernel, np.float32)[:, g * OUT_C:(g + 1) * OUT_C]
            ),
            "thr": np.ascontiguousarray(
                np.asarray(thresholds, np.float32)[:, g * NS:(g + 1) * NS]
            ),
            "mu": mu_c,
        })
    return in_maps


def _assemble(results):
    y = np.empty((T, OUT), np.float32)
    for core in range(NCORES):
        h, g = divmod(core, GRP_WAYS)
        y[h * T_C:(h + 1) * T_C, g * OUT_C:(g + 1) * OUT_C] = results[core]["y"]
    return y.reshape(B, S, OUT)


def run(inputs, **spmd_kwargs):
    """Run on hardware; returns (y, BassKernelResults)."""
    nc = _get_nc()
    in_maps = _make_in_maps(
        inputs["x"], inputs["W_kernel"], inputs["thresholds"],
        inputs["mu"], inputs["out_mu"],
    )
    res = bass_utils.run_bass_kernel_spmd(
        nc, in_maps, core_ids=list(range(NCORES)), **spmd_kwargs
    )
    return _assemble(res.results), res


def kernel(x, W_kernel, thresholds, mu, out_mu, where):
    y, _ = run({
        "x": x, "W_kernel": W_kernel, "thresholds": thresholds,
        "mu": mu, "out_mu": out_mu, "where": where,
    })
    return y


# revision 10
# speedup vs baseline: 2.9180x; 2.9180x over previous
"""Trainium2 Bass kernel for the CWICDense (conditional stripe matmul) module.

Problem (hardcoded shapes):
  x          [2, 512, 4096] f32    tokens T=1024, features I=4096
  W_kernel   [4096, 4096]   f32    viewed as [I, N=32 stripes, Q=128]
  thresholds [4096, 32]     f32
  mu         [4096]         f32    (structurally zero in this module)
  out_mu     [4096]         f32
  where      [2, 512]       bool   (unused by the reference computation)

  y[t, n*Q+q] = sum_i x_off[t,i] * (|x_off[t,i]| >= thresholds[i,n]) * W[i, n*Q+q]
                + out_mu[n*Q+q]

Sharding across 8 NeuronCores: 2-way data parallel over tokens (halves of
512) x 4-way tensor parallel over stripes (groups of 8 stripes = 1024 out
cols). Each core computes y_c [512, 1024].

Per-core device algorithm:
  - PE-transpose x_c to x^T [I on partitions, T free] (128x128 blocks).
  - a = |x^T| once (ACT Abs, exact fp32).
  - per (stripe n, k-tile): z = (a >= thr[:,n]) * x^T with exact fp32
    compares: fused scalar_tensor_tensor on DVE for most k-tiles,
    mask+multiply pair on GPSIMD for the rest.
  - PE matmul (float32r, N=512 moving) accumulating y^T[n-block] in PSUM
    over 32 k-tiles: acc += W[k,n].T @ z.
  - epilogue: ACT adds out_mu (per-partition bias in y^T layout),
    PE-transpose back to [token, outcol] tiles, DMA out.
"""

import sys

if "/opt/trn_rl_repo" not in sys.path:
    sys.path.insert(0, "/opt/trn_rl_repo")

import numpy as np

import concourse.bass as bass
import concourse.mybir as mybir
import concourse.tile as tile
from concourse import bacc, bass_utils
from concourse.masks import make_identity

# ---- problem constants -------------------------------------------------
B, S, I, N, Q = 2, 512, 4096, 32, 128
T = B * S                 # 1024 tokens
OUT = N * Q               # 4096
NCORES = 8
TOK_WAYS = 2              # token halves
GRP_WAYS = 4              # stripe groups
T_C = T // TOK_WAYS       # 512 tokens per core
NS = N // GRP_WAYS        # 8 stripes per core
OUT_C = NS * Q            # 1024 out cols per core
KT = I // 128             # 32 contraction tiles
P = 128

# z-production split within each stripe's 32 k-tiles: the first DVE_K run as
# one fused scalar_tensor_tensor on DVE; the rest run fully on GPSIMD as a
# mask (tensor_scalar is_ge) + multiply (tensor_tensor) pair.
DVE_K = 32

_CACHE = {}


def _build():
    f32 = mybir.dt.float32
    f32r = mybir.dt.float32r
    nc = bacc.Bacc("TRN2", target_bir_lowering=False, debug=False)

    x_d = nc.dram_tensor("x", [T_C, I], f32, kind="ExternalInput").ap()
    w_d = nc.dram_tensor("w", [I, OUT_C], f32, kind="ExternalInput").ap()
    thr_d = nc.dram_tensor("thr", [I, NS], f32, kind="ExternalInput").ap()
    mu_d = nc.dram_tensor("mu", [P, NS], f32, kind="ExternalInput").ap()
    y_d = nc.dram_tensor("y", [T_C, OUT_C], f32, kind="ExternalOutput").ap()

    # strided DRAM views for batched loads
    # x_v[p, c, i] = x[c*128+p, i]
    x_v = x_d.rearrange("(c p) i -> p c i", p=P)
    # w_v[p, k, c] = w[k*128+p, c]
    w_v = w_d.rearrange("(k p) c -> p k c", p=P)

    ge = mybir.AluOpType.is_ge
    mult = mybir.AluOpType.mult

    with tile.TileContext(nc) as tc:
        with (
            tc.tile_pool(name="const", bufs=1) as constp,
            tc.tile_pool(name="xT", bufs=KT) as xTp,
            tc.tile_pool(name="absa", bufs=KT) as ap_,
            tc.tile_pool(name="xnat", bufs=3) as xnatp,
            tc.tile_pool(name="w", bufs=2) as wp,
            tc.tile_pool(name="z", bufs=8) as zp,
            tc.tile_pool(name="m", bufs=3) as mp,
            tc.tile_pool(name="yT", bufs=2) as yTp,
            tc.tile_pool(name="ysb", bufs=2) as ysbp,
            tc.tile_pool(name="tps", bufs=3, space="PSUM") as tpsp,
            tc.tile_pool(name="acc", bufs=3, space="PSUM") as accp,
        ):
            ident = constp.tile([P, P], f32, tag="ident")
            make_identity(nc, ident[:])

            thr_sb = constp.tile([P, KT * NS], f32, tag="thr")
            for k in range(KT):
                nc.sync.dma_start(
                    thr_sb[:, k * NS:(k + 1) * NS], thr_d[k * P:(k + 1) * P, :]
                )
            mu_sb = constp.tile([P, NS], f32, tag="mu")
            nc.sync.dma_start(mu_sb[:], mu_d)

            # ---- phase A: x -> x^T (PE transpose), a = |x^T| ----------
            xT = []
            aT = []
            for k in range(KT):
                xn = xnatp.tile([P, T_C], f32, tag="xnat")
                nc.sync.dma_start(
                    xn[:].rearrange("p (c i) -> p c i", i=P),
                    x_v[:, :, k * P:(k + 1) * P],
                )
                xk = xTp.tile([P, T_C], f32, tag="xT")
                ps = tpsp.tile([P, T_C], f32, tag="tps")
                for c in range(T_C // P):
                    nc.tensor.transpose(
                        ps[:, c * P:(c + 1) * P], xn[:, c * P:(c + 1) * P],
                        ident[:],
                    )
                nc.scalar.copy(xk[:], ps[:])
                ak = ap_.tile([P, T_C], f32, tag="absa")
                nc.scalar.activation(
                    ak[:], xk[:], mybir.ActivationFunctionType.Abs
                )
                xT.append(xk)
                aT.append(ak)

            # ---- phase B: masked stripe matmuls -----------------------
            for n in range(NS):
                # whole W column panel for this stripe in one DMA:
                # wn[p, k*Q+q] = w[k*128+p, n*Q+q]
                wn = wp.tile([P, KT * Q], f32r, tag="w")
                nc.sync.dma_start(
                    wn[:].rearrange("p (k q) -> p k q", q=Q),
                    w_v[:, :, n * Q:(n + 1) * Q].bitcast(f32r),
                )
                acc = accp.tile([P, T_C], f32, tag="acc")
                for k in range(KT):
                    zt = zp.tile([P, T_C], f32r, tag="z")
                    thr_ap = thr_sb[:, k * NS + n:k * NS + n + 1]
                    if k < DVE_K:
                        nc.vector.scalar_tensor_tensor(
                            zt[:], aT[k][:], thr_ap, xT[k][:],
                            op0=ge, op1=mult,
                        )
                    else:
                        mt = mp.tile([P, T_C], f32, tag="m")
                        nc.gpsimd.tensor_scalar(
                            mt[:], aT[k][:], thr_ap, None, op0=ge,
                        )
                        nc.gpsimd.tensor_tensor(
                            zt[:], mt[:], xT[k][:], op=mult
                        )
                    nc.tensor.matmul(
                        acc[:],
                        wn[:, k * Q:(k + 1) * Q],
                        zt[:],
                        start=(k == 0),
                        stop=(k == KT - 1),
                    )
                # epilogue for stripe n: + out_mu (ACT), transpose, store
                yT = yTp.tile([P, T_C], f32, tag="yT")
                nc.scalar.activation(
                    yT[:], acc[:], mybir.ActivationFunctionType.Identity,
                    bias=mu_sb[:, n:n + 1],
                )
                ps2 = tpsp.tile([P, T_C], f32, tag="tps")
                for c in range(T_C // P):
                    nc.tensor.transpose(
                        ps2[:, c * P:(c + 1) * P], yT[:, c * P:(c + 1) * P],
                        ident[:],
                    )
                ysb = ysbp.tile([P, T_C], f32, tag="ysb")
                nc.scalar.copy(ysb[:], ps2[:])
                for c in range(T_C // P):
                    nc.sync.dma_start(
                        y_d[c * P:(c + 1) * P, n * Q:(n + 1) * Q],
                        ysb[:, c * P:(c + 1) * P],
                    )
    nc.compile()
    return nc


def _get_nc():
    if "nc" not in _CACHE:
        _CACHE["nc"] = _build()
    return _CACHE["nc"]


def _make_in_maps(x, W_kernel, thresholds, mu, out_mu):
    xf = np.ascontiguousarray(x, dtype=np.float32).reshape(T, I)
    xf = xf - np.asarray(mu, dtype=np.float32)[None, :]
    in_maps = []
    for core in range(NCORES):
        h, g = divmod(core, GRP_WAYS)
        mu_c = np.ascontiguousarray(
            np.asarray(out_mu, dtype=np.float32)[g * OUT_C:(g + 1) * OUT_C]
            .reshape(NS, P).T
        )
        in_maps.append({
            "x": np.ascontiguousarray(xf[h * T_C:(h + 1) * T_C]),
            "w": np.ascontiguousarray(
                np.asarray(W_kernel, np.float32)[:, g * OUT_C:(g + 1) * OUT_C]
            ),
            "thr": np.ascontiguousarray(
                np.asarray(thresholds, np.float32)[:, g * NS:(g + 1) * NS]
            ),
            "mu": mu_c,
        })
    return in_maps


def _assemble(results):
    y = np.empty((T, OUT), np.float32)
    for core in range(NCORES):
        h, g = divmod(core, GRP_WAYS)
        y[h * T_C:(h + 1) * T_C, g * OUT_C:(g + 1) * OUT_C] = results[core]["y"]
    return y.reshape(B, S, OUT)


def run(inputs, **spmd_kwargs):
    """Run on hardware; returns (y, BassKernelResults)."""
    nc = _get_nc()
    in_maps = _make_in_maps(
        inputs["x"], inputs["W_kernel"], inputs["thresholds"],
        inputs["mu"], inputs["out_mu"],
    )
    res = bass_utils.run_bass_kernel_spmd(
        nc, in_maps, core_ids=list(range(NCORES)), **spmd_kwargs
    )
    return _assemble(res.results), res


def kernel(x, W_kernel, thresholds, mu, out_mu, where):
    y, _ = run({
        "x": x, "W_kernel": W_kernel, "thresholds": thresholds,
        "mu": mu, "out_mu": out_mu, "where": where,
    })
    return y


# revision 12
# speedup vs baseline: 3.0032x; 1.0292x over previous
"""Trainium2 Bass kernel for the CWICDense (conditional stripe matmul) module.

Problem (hardcoded shapes):
  x          [2, 512, 4096] f32    tokens T=1024, features I=4096
  W_kernel   [4096, 4096]   f32    viewed as [I, N=32 stripes, Q=128]
  thresholds [4096, 32]     f32
  mu         [4096]         f32    (structurally zero in this module)
  out_mu     [4096]         f32
  where      [2, 512]       bool   (unused by the reference computation)

  y[t, n*Q+q] = sum_i x_off[t,i] * (|x_off[t,i]| >= thresholds[i,n]) * W[i, n*Q+q]
                + out_mu[n*Q+q]

Sharding across 8 NeuronCores: 8-way tensor parallel over stripes (4 stripes
= 512 out cols per core); every core sees all 1024 tokens. The host passes
x pre-transposed (pure layout prep) so the device does no transposes at all,
and each core returns its y^T block which the host transposes back.

Per-core device algorithm:
  - x^T k-tiles [128 features, 1024 tokens] DMA'd contiguously (resident).
  - per (stripe n, k-tile): one custom DVE instruction computes
      z = select((x >= t_n) | (x <= -t_n), x, 0)
    which equals x * (|x| >= t_n) exactly in fp32 (single input stream).
  - PE matmul (float32r, N=512 moving) accumulating y^T[n-block] in PSUM
    over 32 k-tiles: acc += W[k,n].T @ z, two token-halves per stripe.
  - epilogue: ACT adds out_mu (per-partition bias in y^T layout), DMA out.
"""

import sys

if "/opt/trn_rl_repo" not in sys.path:
    sys.path.insert(0, "/opt/trn_rl_repo")

import numpy as np

import concourse.bass as bass
import concourse.mybir as mybir
import concourse.tile as tile
from concourse import bacc, bass_utils
from concourse import dve_ops as _dve_ops
from concourse.dve_spec import Spec, Src0, C0, C1, Zero, select, lower
from concourse.dve_spec import _has_src1
from concourse.dve_table_gen import dve_ver_for
from concourse.dve_uop import DveOpSpec

# ---- problem constants -------------------------------------------------
B, S, I, N, Q = 2, 512, 4096, 32, 128
T = B * S                 # 1024 tokens
OUT = N * Q               # 4096
NCORES = 8
NS = N // NCORES          # 4 stripes per core
OUT_C = NS * Q            # 512 out cols per core
KT = I // 128             # 32 contraction tiles
P = 128
HF = 2                    # token halves per matmul group (1024 -> 2 x 512)
TH = T // HF              # 512

_CACHE = {}


def _register_gate_op():
    """Register the fused CWIC gate as a custom DVE op:
    out = select((in0 >= s0) | (in0 <= s1), in0, 0) — call with s0 = t,
    s1 = -t to get x * (|x| >= t) with exact fp32 compares."""
    name = "CWIC_GATE_ANT"
    if name in _dve_ops._SUB_OPCODE_FOR_NAME:
        return next(op for op in _dve_ops.OPS if op.name == name)
    spec = Spec(
        body=select((Src0 >= C0) | (Src0 <= C1), Src0, Zero),
        reference=lambda in0, in1, s0, s1, imm2: np.where(
            (in0 >= s0) | (in0 <= s1), in0, 0.0
        ).astype(np.float32),
    )
    row = max(_dve_ops._SUB_OPCODE_FOR_NAME.values()) + 1
    assert row < 0x20
    _dve_ops._SUB_OPCODE_FOR_NAME[name] = row
    shas = {}
    for ver in ("v3",):
        tmp = DveOpSpec(
            name=name, opcode=row, uops=lower(spec, ver=ver),
            rd1_en=_has_src1(spec),
        )
        shas[ver] = tmp.sha(ver)
    op = _dve_ops.DveOp(name, spec, subdim=False, uops_sha=shas)
    _dve_ops.OPS.append(op)
    _dve_ops.CUSTOM_DVE_SPECS[name] = spec
    return op


def _build():
    f32 = mybir.dt.float32
    f32r = mybir.dt.float32r
    gate_op = _register_gate_op()
    nc = bacc.Bacc("TRN2", target_bir_lowering=False, debug=False)

    xT_d = nc.dram_tensor("xT", [I, T], f32, kind="ExternalInput").ap()
    w_d = nc.dram_tensor("w", [I, OUT_C], f32, kind="ExternalInput").ap()
    # thr holds [t | -t]: cols 0..NS-1 are thresholds, NS..2*NS-1 negated
    thr_d = nc.dram_tensor("thr", [I, 2 * NS], f32, kind="ExternalInput").ap()
    mu_d = nc.dram_tensor("mu", [P, NS], f32, kind="ExternalInput").ap()
    yT_d = nc.dram_tensor("yT", [OUT_C, T], f32, kind="ExternalOutput").ap()

    # w_v[p, k, c] = w[k*128+p, c]
    w_v = w_d.rearrange("(k p) c -> p k c", p=P)

    with tile.TileContext(nc) as tc:
        with (
            tc.tile_pool(name="const", bufs=1) as constp,
            tc.tile_pool(name="xT", bufs=KT) as xTp,
            tc.tile_pool(name="w", bufs=2) as wp,
            tc.tile_pool(name="z", bufs=5) as zp,
            tc.tile_pool(name="yT", bufs=4) as yTp,
            tc.tile_pool(name="acc", bufs=4, space="PSUM") as accp,
        ):
            thr_sb = constp.tile([P, KT * 2 * NS], f32, tag="thr")
            for k in range(KT):
                nc.sync.dma_start(
                    thr_sb[:, k * 2 * NS:(k + 1) * 2 * NS],
                    thr_d[k * P:(k + 1) * P, :],
                )
            mu_sb = constp.tile([P, NS], f32, tag="mu")
            nc.sync.dma_start(mu_sb[:], mu_d)

            xT = []
            for k in range(KT):
                xk = xTp.tile([P, T], f32, tag="xT")
                nc.sync.dma_start(xk[:], xT_d[k * P:(k + 1) * P, :])
                xT.append(xk)

            for n in range(NS):
                # whole W column panel for this stripe in one DMA:
                # wn[p, k*Q+q] = w[k*128+p, n*Q+q]
                wn = wp.tile([P, KT * Q], f32r, tag="w")
                nc.sync.dma_start(
                    wn[:].rearrange("p (k q) -> p k q", q=Q),
                    w_v[:, :, n * Q:(n + 1) * Q].bitcast(f32r),
                )
                accs = [
                    accp.tile([P, TH], f32, tag="acc", name=f"acc{h}")
                    for h in range(HF)
                ]
                for k in range(KT):
                    zt = zp.tile([P, T], f32r, tag="z")
                    nc.vector._custom_dve(
                        gate_op,
                        out=zt[:],
                        in0=xT[k][:],
                        s0=thr_sb[:, k * 2 * NS + n:k * 2 * NS + n + 1],
                        s1=thr_sb[:, k * 2 * NS + NS + n:k * 2 * NS + NS + n + 1],
                    )
                    for h in range(HF):
                        nc.tensor.matmul(
                            accs[h][:],
                            wn[:, k * Q:(k + 1) * Q],
                            zt[:, h * TH:(h + 1) * TH],
                            start=(k == 0),
                            stop=(k == KT - 1),
                        )
                # epilogue: + out_mu (per-partition in y^T layout), DMA out
                for h in range(HF):
                    yt = yTp.tile([P, TH], f32, tag="yT")
                    nc.scalar.activation(
                        yt[:], accs[h][:],
                        mybir.ActivationFunctionType.Identity,
                        bias=mu_sb[:, n:n + 1],
                    )
                    nc.sync.dma_start(
                        yT_d[n * P:(n + 1) * P, h * TH:(h + 1) * TH], yt[:]
                    )
    nc.compile()
    return nc


def _get_nc():
    if "nc" not in _CACHE:
        _CACHE["nc"] = _build()
    return _CACHE["nc"]


def _make_in_maps(x, W_kernel, thresholds, mu, out_mu):
    xf = np.asarray(x, dtype=np.float32).reshape(T, I)
    xf = xf - np.asarray(mu, dtype=np.float32)[None, :]
    xT = np.ascontiguousarray(xf.T)
    W = np.asarray(W_kernel, np.float32)
    thr = np.asarray(thresholds, np.float32)
    omu = np.asarray(out_mu, np.float32)
    in_maps = []
    for g in range(NCORES):
        thr_c = thr[:, g * NS:(g + 1) * NS]
        in_maps.append({
            "xT": xT,
            "w": np.ascontiguousarray(W[:, g * OUT_C:(g + 1) * OUT_C]),
            "thr": np.ascontiguousarray(
                np.concatenate([thr_c, -thr_c], axis=1)
            ),
            "mu": np.ascontiguousarray(
                omu[g * OUT_C:(g + 1) * OUT_C].reshape(NS, P).T
            ),
        })
    return in_maps


def _assemble(results):
    yT = np.concatenate([results[g]["yT"] for g in range(NCORES)], axis=0)
    return np.ascontiguousarray(yT.T).reshape(B, S, OUT)


def run(inputs, **spmd_kwargs):
    """Run on hardware; returns (y, BassKernelResults)."""
    nc = _get_nc()
    in_maps = _make_in_maps(
        inputs["x"], inputs["W_kernel"], inputs["thresholds"],
        inputs["mu"], inputs["out_mu"],
    )
    res = bass_utils.run_bass_kernel_spmd(
        nc, in_maps, core_ids=list(range(NCORES)), **spmd_kwargs
    )
    return _assemble(res.results), res


def kernel(x, W_kernel, thresholds, mu, out_mu, where):
    y, _ = run({
        "x": x, "W_kernel": W_kernel, "thresholds": thresholds,
        "mu": mu, "out_mu": out_mu, "where": where,
    })
    return y


# revision 17
# speedup vs baseline: 3.6634x; 1.2198x over previous
"""Trainium2 Bass kernel for the CWICDense (conditional stripe matmul) module.

Problem (hardcoded shapes):
  x          [2, 512, 4096] f32    tokens T=1024, features I=4096
  W_kernel   [4096, 4096]   f32    viewed as [I, N=32 stripes, Q=128]
  thresholds [4096, 32]     f32
  mu         [4096]         f32    (structurally zero in this module)
  out_mu     [4096]         f32
  where      [2, 512]       bool   (unused by the reference computation)

  y[t, n*Q+q] = sum_i x_off[t,i] * (|x_off[t,i]| >= thresholds[i,n]) * W[i, n*Q+q]
                + out_mu[n*Q+q]

Sharding across 8 NeuronCores: 8-way tensor parallel over stripes (4 stripes
= 512 out cols per core); every core sees all 1024 tokens. The host passes
x pre-transposed (pure layout prep) so the device does no transposes at all,
and each core returns its y^T block which the host transposes back.

Per-core device algorithm:
  - x^T k-tiles [128 features, 1024 tokens] DMA'd contiguously (resident).
  - per (stripe n, k-tile): one custom DVE instruction computes
      z = select((x >= t_n) | (x <= -t_n), x, 0)
    which equals x * (|x| >= t_n) exactly in fp32 (single input stream).
  - PE matmul (float32r, N=512 moving) accumulating y^T[n-block] in PSUM
    over 32 k-tiles: acc += W[k,n].T @ z, two token-halves per stripe.
  - epilogue: ACT adds out_mu (per-partition bias in y^T layout), DMA out.
"""

import sys

if "/opt/trn_rl_repo" not in sys.path:
    sys.path.insert(0, "/opt/trn_rl_repo")

import numpy as np

import concourse.bass as bass
import concourse.mybir as mybir
import concourse.tile as tile
from concourse import bacc, bass_utils
from concourse import dve_ops as _dve_ops
from concourse.dve_spec import Spec, Src0, C0, C1, Zero, select, lower
from concourse.dve_spec import _has_src1
from concourse.dve_table_gen import dve_ver_for
from concourse.dve_uop import DveOpSpec

# ---- problem constants -------------------------------------------------
B, S, I, N, Q = 2, 512, 4096, 32, 128
T = B * S                 # 1024 tokens
OUT = N * Q               # 4096
NCORES = 8
NS = N // NCORES          # 4 stripes per core
OUT_C = NS * Q            # 512 out cols per core
KT = I // 128             # 32 contraction tiles
P = 128
HF = 2                    # token halves per matmul group (1024 -> 2 x 512)
TH = T // HF              # 512

_CACHE = {}


def _register_gate_op():
    """Register the fused CWIC gate as a custom DVE op:
    out = select((in0 >= s0) | (in0 <= s1), in0, 0) — call with s0 = t,
    s1 = -t to get x * (|x| >= t) with exact fp32 compares."""
    name = "CWIC_GATE_ANT"
    if name in _dve_ops._SUB_OPCODE_FOR_NAME:
        return next(op for op in _dve_ops.OPS if op.name == name)
    spec = Spec(
        body=select((Src0 >= C0) | (Src0 <= C1), Src0, Zero),
        reference=lambda in0, in1, s0, s1, imm2: np.where(
            (in0 >= s0) | (in0 <= s1), in0, 0.0
        ).astype(np.float32),
    )
    row = max(_dve_ops._SUB_OPCODE_FOR_NAME.values()) + 1
    assert row < 0x20
    _dve_ops._SUB_OPCODE_FOR_NAME[name] = row
    shas = {}
    for ver in ("v3",):
        tmp = DveOpSpec(
            name=name, opcode=row, uops=lower(spec, ver=ver),
            rd1_en=_has_src1(spec),
        )
        shas[ver] = tmp.sha(ver)
    op = _dve_ops.DveOp(name, spec, subdim=False, uops_sha=shas)
    _dve_ops.OPS.append(op)
    _dve_ops.CUSTOM_DVE_SPECS[name] = spec
    return op


def _build():
    f32 = mybir.dt.float32
    f32r = mybir.dt.float32r
    gate_op = _register_gate_op()
    nc = bacc.Bacc("TRN2", target_bir_lowering=False, debug=False)

    xT_d = nc.dram_tensor("xT", [I, T], f32, kind="ExternalInput").ap()
    w_d = nc.dram_tensor("w", [I, OUT_C], f32, kind="ExternalInput").ap()
    # thr holds [t | -t]: cols 0..NS-1 are thresholds, NS..2*NS-1 negated
    thr_d = nc.dram_tensor("thr", [I, 2 * NS], f32, kind="ExternalInput").ap()
    mu_d = nc.dram_tensor("mu", [P, NS], f32, kind="ExternalInput").ap()
    yT_d = nc.dram_tensor("yT", [OUT_C, T], f32, kind="ExternalOutput").ap()

    # w_v[p, k, c] = w[k*128+p, c]
    w_v = w_d.rearrange("(k p) c -> p k c", p=P)

    with tile.TileContext(nc) as tc:
        with (
            tc.tile_pool(name="const", bufs=1) as constp,
            tc.tile_pool(name="xT", bufs=KT) as xTp,
            tc.tile_pool(name="thr", bufs=KT) as thrp,
            tc.tile_pool(name="w", bufs=2) as wp,
            tc.tile_pool(name="z", bufs=6) as zp,
            tc.tile_pool(name="yT", bufs=2) as yTp,
            tc.tile_pool(name="acc", bufs=4, space="PSUM") as accp,
            tc.tile_pool(name="warm", bufs=1, space="PSUM") as warmp,
        ):
            mu_sb = constp.tile([P, NS], f32, tag="mu")
            nc.sync.dma_start(mu_sb[:], mu_d)

            # interleave per-k threshold + x loads so z(n=0, k=0) can start
            # as soon as the first pair lands (per-tile dep granularity)
            xT = []
            thrT = []

            def load_pair(k):
                tk = thrp.tile([P, 2 * NS], f32, tag="thr", name=f"thr{k}")
                nc.sync.dma_start(tk[:], thr_d[k * P:(k + 1) * P, :])
                xk = xTp.tile([P, T], f32, tag="xT", name=f"xk{k}")
                nc.sync.dma_start(xk[:], xT_d[k * P:(k + 1) * P, :])
                xT.append(xk)
                thrT.append(tk)

            for k in range(5):
                load_pair(k)

            # HAM warm-up: a short burst of throwaway matmuls keyed on a DMA
            # that lands mid x-stream, so the PE clock is at 2.4 GHz just as
            # real matmuls arrive (a cold PE runs at 1.2 GHz).
            warmsrc = constp.tile([P, TH], f32r, tag="warmsrc")
            nc.sync.dma_start(warmsrc[:], xT_d[P:2 * P, 0:TH].bitcast(f32r))
            warm = warmp.tile([P, TH], f32, tag="warm")
            for _ in range(6):
                nc.tensor.matmul(
                    warm[:],
                    warmsrc[:, 0:P],
                    warmsrc[:],
                    start=True,
                    stop=True,
                )

            for k in range(5, KT):
                load_pair(k)

            for n in range(NS):
                # whole W column panel for this stripe in one DMA:
                # wn[p, k*Q+q] = w[k*128+p, n*Q+q]
                # W panels go through the (otherwise idle) scalar engine's
                # DMA queue so they don't serialize behind the x loads
                wn = wp.tile([P, KT * Q], f32r, tag="w")
                nc.scalar.dma_start(
                    wn[:].rearrange("p (k q) -> p k q", q=Q),
                    w_v[:, :, n * Q:(n + 1) * Q].bitcast(f32r),
                )
                accs = [
                    accp.tile([P, TH], f32, tag="acc", name=f"acc{h}")
                    for h in range(HF)
                ]
                for k in range(KT):
                    zt = zp.tile([P, T], f32r, tag="z")
                    nc.vector._custom_dve(
                        gate_op,
                        out=zt[:],
                        in0=xT[k][:],
                        s0=thrT[k][:, n:n + 1],
                        s1=thrT[k][:, NS + n:NS + n + 1],
                    )
                    for h in range(HF):
                        nc.tensor.matmul(
                            accs[h][:],
                            wn[:, k * Q:(k + 1) * Q],
                            zt[:, h * TH:(h + 1) * TH],
                            start=(k == 0),
                            stop=(k == KT - 1),
                        )
                # epilogue: + out_mu (per-partition in y^T layout), DMA out
                for h in range(HF):
                    yt = yTp.tile([P, TH], f32, tag="yT")
                    nc.scalar.activation(
                        yt[:], accs[h][:],
                        mybir.ActivationFunctionType.Identity,
                        bias=mu_sb[:, n:n + 1],
                    )
                    nc.sync.dma_start(
                        yT_d[n * P:(n + 1) * P, h * TH:(h + 1) * TH], yt[:]
                    )
    nc.compile()
    return nc


def _get_nc():
    if "nc" not in _CACHE:
        _CACHE["nc"] = _build()
    return _CACHE["nc"]


def _make_in_maps(x, W_kernel, thresholds, mu, out_mu):
    xf = np.asarray(x, dtype=np.float32).reshape(T, I)
    xf = xf - np.asarray(mu, dtype=np.float32)[None, :]
    xT = np.ascontiguousarray(xf.T)
    W = np.asarray(W_kernel, np.float32)
    thr = np.asarray(thresholds, np.float32)
    omu = np.asarray(out_mu, np.float32)
    in_maps = []
    for g in range(NCORES):
        thr_c = thr[:, g * NS:(g + 1) * NS]
        in_maps.append({
            "xT": xT,
            "w": np.ascontiguousarray(W[:, g * OUT_C:(g + 1) * OUT_C]),
            "thr": np.ascontiguousarray(
                np.concatenate([thr_c, -thr_c], axis=1)
            ),
            "mu": np.ascontiguousarray(
                omu[g * OUT_C:(g + 1) * OUT_C].reshape(NS, P).T
            ),
        })
    return in_maps


def _assemble(results):
    yT = np.concatenate([results[g]["yT"] for g in range(NCORES)], axis=0)
    return np.ascontiguousarray(yT.T).reshape(B, S, OUT)


def run(inputs, **spmd_kwargs):
    """Run on hardware; returns (y, BassKernelResults)."""
    nc = _get_nc()
    in_maps = _make_in_maps(
        inputs["x"], inputs["W_kernel"], inputs["thresholds"],
        inputs["mu"], inputs["out_mu"],
    )
    res = bass_utils.run_bass_kernel_spmd(
        nc, in_maps, core_ids=list(range(NCORES)), **spmd_kwargs
    )
    return _assemble(res.results), res


def kernel(x, W_kernel, thresholds, mu, out_mu, where):
    y, _ = run({
        "x": x, "W_kernel": W_kernel, "thresholds": thresholds,
        "mu": mu, "out_mu": out_mu, "where": where,
    })
    return y
